# revision 13
# baseline (speedup 1.0000x reference)
"""Multi-head self-attention (B=2, S=2048, D=1024, H=16) on 8 TRN2 NeuronCores.

Tensor-parallel over heads: each core owns 2 heads. Accepts FULL inputs,
returns FULL output. Host pre-transposes x and slices per-head weights;
each core computes qkv -> per-head LayerNorm -> attention -> partial
output projection (over its 128 embed dims); host sums the 8 partials
and adds the projection bias.
"""

import os
import sys

import numpy as np

for _p in ("/opt/trn_rl_repo", "/root/.axon_site/_ro/trn_rl_repo"):
    if os.path.isdir(_p) and _p not in sys.path:
        sys.path.insert(0, _p)
        break

import concourse.bass as bass  # noqa: E402
import concourse.bacc as bacc  # noqa: E402
import concourse.tile as tile  # noqa: E402
from concourse import mybir  # noqa: E402
from concourse.bass_utils import run_bass_kernel_spmd  # noqa: E402

F32 = mybir.dt.float32
F32R = mybir.dt.float32r
BF16 = mybir.dt.bfloat16
AF = mybir.ActivationFunctionType
ALU = mybir.AluOpType

NCORES = 8
D = 1024
H = 16
HD = 64
HPC = H // NCORES          # heads per core = 2
DPC = HPC * HD             # embed dims per core = 128
EPS = 1e-5


def build_nc(B, S, affine):
    """Build the SPMD Bass program for one core (same program, 8 cores)."""
    T = B * S                      # total token columns
    NTB = T // 128                 # 128-token blocks
    QC = S // 512                  # q-chunks per batch
    KB = S // 128                  # k-blocks per batch
    KCH = D // 128                 # contraction chunks (8)
    SCALE = 1.0 / np.sqrt(HD)

    nc = bacc.Bacc(
        "TRN2",
        target_bir_lowering=False,
        debug=False,
        enable_asserts=True,
        num_devices=NCORES,
    )

    xT = nc.dram_tensor("xT", [D, T], BF16, kind="ExternalInput").ap()
    wq = nc.dram_tensor("wt_qkv", [D, 3 * DPC], BF16, kind="ExternalInput").ap()
    bq = nc.dram_tensor("b_qkv_s", [1, 3 * DPC], BF16, kind="ExternalInput").ap()
    wp = nc.dram_tensor("wt_proj", [DPC, D], BF16, kind="ExternalInput").ap()
    onesb = nc.dram_tensor("c_onesb", [1, 128], BF16, kind="ExternalInput").ap()
    if affine:
        gb = nc.dram_tensor("c_gb", [128, 4, HD], F32, kind="ExternalInput").ap()
    outp = nc.dram_tensor("outp", [T, D], BF16, kind="ExternalOutput").ap()

    from contextlib import ExitStack

    with tile.TileContext(nc) as tc, ExitStack() as stack:
        const = stack.enter_context(tc.tile_pool(name="const", bufs=1))
        persist = stack.enter_context(tc.tile_pool(name="persist", bufs=1))

        wq_sb = const.tile([128, KCH, 3 * DPC], BF16, tag="wq")
        nc.sync.dma_start(
            out=wq_sb, in_=wq.rearrange("(c p) n -> p c n", p=128)
        )
        wp_sb = const.tile([DPC, D], BF16, tag="wp")
        nc.sync.dma_start(out=wp_sb, in_=wp)
        bq_sb = const.tile([1, 3 * DPC], BF16, tag="bq")
        nc.sync.dma_start(out=bq_sb, in_=bq)
        onesb_sb = const.tile([1, 128], BF16, tag="onesb")
        nc.sync.dma_start(out=onesb_sb, in_=onesb)
        eps_sb = const.tile([128, 1], F32, tag="eps")
        nc.vector.memset(eps_sb, EPS)

        if affine:
            gb_sb = const.tile([128, 4, HD], F32, tag="gb")
            nc.sync.dma_start(out=gb_sb, in_=gb)

        # whole x^T resident in SBUF; loaded in 8 token-chunks so the first
        # qkv block only waits on chunk 0
        xt_all = const.tile([128, KCH, T], BF16, tag="xt")
        for n in range(T // 512):
            nc.sync.dma_start(
                out=xt_all[:, :, n * 512 : (n + 1) * 512],
                in_=xT.rearrange("(c p) t -> p c t", p=128)[
                    :, :, n * 512 : (n + 1) * 512
                ],
            )

        # persistent intermediates
        qT = persist.tile([128, T], BF16, tag="qT")     # [2h*64, tok] LN'd q^T
        kT = persist.tile([128, T], BF16, tag="kT")
        vO = persist.tile([128, HPC, NTB, HD + 1], BF16, tag="vO")
        aT = persist.tile([128, T], BF16, tag="aT")     # attention out^T
        nc.vector.memset(vO[:, :, :, HD : HD + 1], 1.0)

        # ---------------- Phase 1: qkv + LayerNorm + transpose ----------
        with (
            tc.tile_pool(name="qkv_ps", bufs=4, space="PSUM") as qkv_ps,
            tc.tile_pool(name="stage1", bufs=4) as stage1,
            tc.tile_pool(name="stats", bufs=4) as stats_pool,
        ):
            for tb in range(NTB):
                ps = qkv_ps.tile([128, 3 * DPC], F32, tag="ps")
                nc.tensor.matmul(
                    ps,
                    lhsT=onesb_sb[0:1, 0:128],
                    rhs=bq_sb,
                    start=True,
                    stop=False,
                )
                for k in range(KCH):
                    nc.tensor.matmul(
                        ps,
                        lhsT=xt_all[:, k, tb * 128 : (tb + 1) * 128],
                        rhs=wq_sb[:, k, :],
                        start=False,
                        stop=(k == KCH - 1),
                    )
                # LayerNorm over each head's 64 dims of q and k
                qk = ps[:, 0 : 2 * DPC].rearrange("p (g d) -> p g d", d=HD)
                st = stats_pool.tile([128, 4, 6], F32, tag="st")
                mv = stats_pool.tile([128, 4, 2], F32, tag="mv")
                for g in range(4):
                    nc.vector.bn_stats(out=st[:, g, :], in_=qk[:, g, :])
                    nc.vector.bn_aggr(out=mv[:, g, :], in_=st[:, g, :])
                rstd = stats_pool.tile([128, 4], F32, tag="rstd")
                nc.scalar.activation(
                    out=rstd, in_=mv[:, :, 1], func=AF.Sqrt, bias=eps_sb
                )
                nc.vector.reciprocal(out=rstd, in_=rstd)
                # nmr = -mu * rstd (bias for the ACT-side LN apply)
                nmr = stats_pool.tile([128, 4], F32, tag="nmr")
                nc.vector.scalar_tensor_tensor(
                    out=nmr,
                    in0=mv[:, :, 0],
                    scalar=-1.0,
                    in1=rstd,
                    op0=ALU.mult,
                    op1=ALU.mult,
                )
                qn = stage1.tile([128, 128], BF16, tag="qn")
                kn = stage1.tile([128, 128], BF16, tag="kn")
                for g in range(4):
                    dst = qn if g < 2 else kn
                    dsl = dst[:, (g % 2) * HD : (g % 2 + 1) * HD]
                    if g < 2:
                        # q groups on ACT: (x - mu)*rstd == x*rstd + (-mu*rstd)
                        nc.scalar.activation(
                            out=dsl,
                            in_=qk[:, g, :],
                            func=AF.Identity,
                            scale=rstd[:, g : g + 1],
                            bias=nmr[:, g : g + 1],
                        )
                    else:
                        # k groups on DVE (balances ACT vs DVE load)
                        nc.vector.tensor_scalar(
                            out=dsl,
                            in0=qk[:, g, :],
                            scalar1=mv[:, g, 0:1],
                            scalar2=rstd[:, g : g + 1],
                            op0=ALU.subtract,
                            op1=ALU.mult,
                        )
                    if affine:
                        nc.vector.tensor_mul(dsl, dsl, gb_sb[:, 2 * (g // 2), :])
                        nc.vector.tensor_add(
                            dsl, dsl, gb_sb[:, 2 * (g // 2) + 1, :]
                        )
                # v (+ ones col already set)
                nc.vector.tensor_copy(
                    out=vO[:, :, tb, 0:HD],
                    in_=ps[:, 2 * DPC :].rearrange("p (h d) -> p h d", d=HD),
                )
                # transpose q,k into [dim, token] layout via DMA xbar
                ts = slice(tb * 128, (tb + 1) * 128)
                nc.sync.dma_start_transpose(out=qT[:, ts], in_=qn)
                nc.sync.dma_start_transpose(out=kT[:, ts], in_=kn)

        # ---------------- Phase 2: attention + projection ----------------
        with (
            tc.tile_pool(name="sc_ps", bufs=2, space="PSUM") as sc_ps,
            tc.tile_pool(name="o_ps", bufs=2, space="PSUM") as o_ps,
            tc.tile_pool(name="exps", bufs=4) as exps,
            tc.tile_pool(name="stage2", bufs=2) as stage2,
            tc.tile_pool(name="ostage", bufs=3) as ostage,
        ):
            for b in range(B):
                for qc in range(QC):
                    cols = slice(b * S + qc * 512, b * S + (qc + 1) * 512)
                    oom = o_ps.tile(
                        [HD + 1, HPC, 512], F32, tag="o", name="oom"
                    )
                    for kb in range(KB):
                        gkb = b * KB + kb
                        ks = slice(gkb * 128, (gkb + 1) * 128)
                        # two heads' score matmuls live at partition bases
                        # 0/64 -> disjoint PE row groups run concurrently;
                        # one 1024-wide exp covers both heads
                        scp = sc_ps.tile(
                            [128, HPC, 512], F32, tag="s", name="scp"
                        )
                        for h in range(HPC):
                            hp = slice(h * HD, (h + 1) * HD)
                            nc.tensor.matmul(
                                scp[:, h, :],
                                lhsT=kT[hp, ks],
                                rhs=qT[hp, cols],
                                start=True,
                                stop=True,
                            )
                        ex = exps.tile(
                            [128, HPC, 512], BF16, tag="ex", name="ex"
                        )
                        nc.scalar.activation(
                            out=ex, in_=scp, func=AF.Exp, scale=SCALE
                        )
                        for h in range(HPC):
                            nc.tensor.matmul(
                                oom[:, h, :],
                                lhsT=vO[:, h, gkb, :],
                                rhs=ex[:, h, :],
                                start=(kb == 0),
                                stop=(kb == KB - 1),
                            )
                    # 1/denominator on DVE (fast 18-bit custom op), broadcast
                    # across partitions on the idle gpsimd engine (ACT stays
                    # pure-Exp: no activation-table reloads in the phase)
                    dn = stage2.tile([1, HPC, 512], F32, tag="dn", name="dn")
                    nc.vector.tensor_copy(out=dn, in_=oom[HD : HD + 1, :, :])
                    rc = stage2.tile([1, HPC, 512], F32, tag="rc", name="rc")
                    nc.vector.reciprocal_approx_fast(out=rc, in_=dn)
                    rbs = stage2.tile(
                        [128, HPC, 512], F32, tag="rbs", name="rbs"
                    )
                    nc.gpsimd.partition_broadcast(rbs, rc)
                    for h in range(HPC):
                        nc.vector.tensor_mul(
                            aT[h * HD : (h + 1) * HD, cols],
                            oom[0:HD, h, :],
                            rbs[h * HD : (h + 1) * HD, h, :],
                        )
                    # fused partial projection for the 4 token blocks of
                    # this q-chunk; PSUM evicted straight to DRAM by DMA
                    # (projection bias is added on the host)
                    for tbl in range(4):
                        tb = (b * QC + qc) * 4 + tbl
                        rows = slice(tb * 128, (tb + 1) * 128)
                        ob = ostage.tile([128, D], BF16, tag="ob")
                        for nn in range(D // 512):
                            # projection PSUM shares the score pool's banks
                            pps = sc_ps.tile(
                                [128, HPC, 512], F32, tag="s", name="pps"
                            )[:, 0, :]
                            nc.tensor.matmul(
                                pps,
                                lhsT=aT[:, rows],
                                rhs=wp_sb[:, nn * 512 : (nn + 1) * 512],
                                start=True,
                                stop=True,
                            )
                            nc.vector.tensor_copy(
                                out=ob[:, nn * 512 : (nn + 1) * 512], in_=pps
                            )
                        nc.sync.dma_start(out=outp[rows, :], in_=ob)

    nc.compile()
    return nc


def make_in_maps(x, w_qkv, b_qkv, w_proj, q_gamma, q_beta, k_gamma, k_beta,
                 affine):
    B, S, _ = x.shape
    T = B * S
    xT = np.ascontiguousarray(x.reshape(T, D).T)
    import ml_dtypes
    bf = ml_dtypes.bfloat16
    in_maps = []
    for c in range(NCORES):
        rs = slice(c * DPC, (c + 1) * DPC)
        w_slice = np.concatenate(
            [w_qkv[rs], w_qkv[D:2 * D][rs.start:rs.stop], w_qkv[2 * D:][rs.start:rs.stop]],
            axis=0,
        )  # [384, 1024]
        b_slice = np.concatenate(
            [b_qkv[rs], b_qkv[D:2 * D][rs.start:rs.stop], b_qkv[2 * D:][rs.start:rs.stop]]
        )[None, :]  # [1, 384]
        m = {
            "xT": xT.astype(bf),
            "wt_qkv": np.ascontiguousarray(w_slice.T).astype(bf),
            "b_qkv_s": np.ascontiguousarray(b_slice).astype(bf),
            "wt_proj": np.ascontiguousarray(w_proj[:, rs].T).astype(bf),
            "c_onesb": np.ones((1, 128), bf),
        }
        if affine:
            gb = np.stack([q_gamma, q_beta, k_gamma, k_beta])  # [4, 64]
            m["c_gb"] = np.ascontiguousarray(
                np.broadcast_to(gb[None], (128, 4, HD)).astype(np.float32)
            )
        in_maps.append(m)
    return in_maps


_NC_CACHE = {}

LAST_RESULTS = None


def kernel(x, w_qkv, b_qkv, w_proj, b_proj, q_gamma, q_beta, k_gamma, k_beta,
           **unused):
    global LAST_RESULTS
    x = np.asarray(x, np.float32)
    w_qkv = np.asarray(w_qkv, np.float32)
    b_qkv = np.asarray(b_qkv, np.float32)
    w_proj = np.asarray(w_proj, np.float32)
    b_proj = np.asarray(b_proj, np.float32)
    q_gamma = np.asarray(q_gamma, np.float32)
    q_beta = np.asarray(q_beta, np.float32)
    k_gamma = np.asarray(k_gamma, np.float32)
    k_beta = np.asarray(k_beta, np.float32)

    B, S, _ = x.shape
    affine = not (
        np.all(q_gamma == 1) and np.all(k_gamma == 1)
        and np.all(q_beta == 0) and np.all(k_beta == 0)
    )
    key = (B, S, affine)
    if key not in _NC_CACHE:
        _NC_CACHE[key] = build_nc(B, S, affine)
    nc = _NC_CACHE[key]

    in_maps = make_in_maps(
        x, w_qkv, b_qkv, w_proj, q_gamma, q_beta, k_gamma, k_beta, affine
    )
    trace = bool(int(os.environ.get("BASS_KERNEL_TRACE", "0")))
    res = run_bass_kernel_spmd(
        nc, in_maps, core_ids=list(range(NCORES)), trace=trace
    )
    LAST_RESULTS = res
    acc = np.zeros((B * S, D), np.float32)
    for r in res.results:
        acc += np.asarray(r["outp"], np.float32)
    acc += b_proj[None, :]
    return acc.reshape(B, S, D)


# revision 15
# speedup vs baseline: 1.1290x; 1.1290x over previous
"""Multi-head self-attention (B=2, S=2048, D=1024, H=16) on 8 TRN2 NeuronCores.

Tensor-parallel over heads: each core owns 2 heads. Accepts FULL inputs,
returns FULL output. Host pre-transposes x and slices per-head weights;
each core computes qkv -> per-head LayerNorm -> attention -> partial
output projection (over its 128 embed dims); host sums the 8 partials
and adds the projection bias.
"""

import os
import sys

import numpy as np

for _p in ("/opt/trn_rl_repo", "/root/.axon_site/_ro/trn_rl_repo"):
    if os.path.isdir(_p) and _p not in sys.path:
        sys.path.insert(0, _p)
        break

import concourse.bass as bass  # noqa: E402
import concourse.bacc as bacc  # noqa: E402
import concourse.tile as tile  # noqa: E402
from concourse import mybir  # noqa: E402
from concourse.bass_utils import run_bass_kernel_spmd  # noqa: E402

F32 = mybir.dt.float32
F32R = mybir.dt.float32r
BF16 = mybir.dt.bfloat16
AF = mybir.ActivationFunctionType
ALU = mybir.AluOpType

NCORES = 8
D = 1024
H = 16
HD = 64
HPC = H // NCORES          # heads per core = 2
DPC = HPC * HD             # embed dims per core = 128
EPS = 1e-5


def build_nc(B, S, affine):
    """Build the SPMD Bass program for one core (same program, 8 cores)."""
    T = B * S                      # total token columns
    NTB = T // 128                 # 128-token blocks
    QC = S // 512                  # q-chunks per batch
    KB = S // 128                  # k-blocks per batch
    KCH = D // 128                 # contraction chunks (8)
    SCALE = 1.0 / np.sqrt(HD)

    nc = bacc.Bacc(
        "TRN2",
        target_bir_lowering=False,
        debug=False,
        enable_asserts=True,
        num_devices=NCORES,
    )

    xT = nc.dram_tensor("xT", [D, T], BF16, kind="ExternalInput").ap()
    wq = nc.dram_tensor("wt_qkv", [D, 3 * DPC], BF16, kind="ExternalInput").ap()
    bq = nc.dram_tensor("b_qkv_s", [1, 3 * DPC], BF16, kind="ExternalInput").ap()
    wp = nc.dram_tensor("wt_proj", [DPC, D], BF16, kind="ExternalInput").ap()
    onesb = nc.dram_tensor("c_onesb", [1, 128], BF16, kind="ExternalInput").ap()
    if affine:
        gb = nc.dram_tensor("c_gb", [128, 4, HD], F32, kind="ExternalInput").ap()
    outp = nc.dram_tensor("outp", [T, D], BF16, kind="ExternalOutput").ap()

    from contextlib import ExitStack

    with tile.TileContext(nc) as tc, ExitStack() as stack:
        const = stack.enter_context(tc.tile_pool(name="const", bufs=1))
        persist = stack.enter_context(tc.tile_pool(name="persist", bufs=1))

        wq_sb = const.tile([128, KCH, 3 * DPC], BF16, tag="wq")
        nc.sync.dma_start(
            out=wq_sb, in_=wq.rearrange("(c p) n -> p c n", p=128)
        )
        wp_sb = const.tile([DPC, D], BF16, tag="wp")
        nc.sync.dma_start(out=wp_sb, in_=wp)
        bq_sb = const.tile([1, 3 * DPC], BF16, tag="bq")
        nc.sync.dma_start(out=bq_sb, in_=bq)
        onesb_sb = const.tile([1, 128], BF16, tag="onesb")
        nc.sync.dma_start(out=onesb_sb, in_=onesb)
        eps_sb = const.tile([128, 1], F32, tag="eps")
        nc.vector.memset(eps_sb, EPS)

        if affine:
            gb_sb = const.tile([128, 4, HD], F32, tag="gb")
            nc.sync.dma_start(out=gb_sb, in_=gb)

        # whole x^T resident in SBUF; loaded in 8 token-chunks so the first
        # qkv block only waits on chunk 0
        xt_all = const.tile([128, KCH, T], BF16, tag="xt")
        for n in range(T // 512):
            nc.sync.dma_start(
                out=xt_all[:, :, n * 512 : (n + 1) * 512],
                in_=xT.rearrange("(c p) t -> p c t", p=128)[
                    :, :, n * 512 : (n + 1) * 512
                ],
            )

        # persistent intermediates
        qT = persist.tile([128, T], BF16, tag="qT")     # [2h*64, tok] LN'd q^T
        kT = persist.tile([128, T], BF16, tag="kT")
        vO = persist.tile([128, HPC, NTB, HD + 1], BF16, tag="vO")
        aT = persist.tile([128, T], BF16, tag="aT")     # attention out^T
        nc.vector.memset(vO[:, :, :, HD : HD + 1], 1.0)

        # ---------------- Phase 1: qkv + LayerNorm + transpose ----------
        with (
            tc.tile_pool(name="qkv_ps", bufs=6, space="PSUM") as qkv_ps,
            tc.tile_pool(name="stage1", bufs=6) as stage1,
            tc.tile_pool(name="stats", bufs=6) as stats_pool,
        ):
            for tb in range(NTB):
                ps = qkv_ps.tile([128, 3 * DPC], F32, tag="ps")
                nc.tensor.matmul(
                    ps,
                    lhsT=onesb_sb[0:1, 0:128],
                    rhs=bq_sb,
                    start=True,
                    stop=False,
                )
                for k in range(KCH):
                    nc.tensor.matmul(
                        ps,
                        lhsT=xt_all[:, k, tb * 128 : (tb + 1) * 128],
                        rhs=wq_sb[:, k, :],
                        start=False,
                        stop=(k == KCH - 1),
                    )
                # LayerNorm over each head's 64 dims of q and k
                qk = ps[:, 0 : 2 * DPC].rearrange("p (g d) -> p g d", d=HD)
                st = stats_pool.tile([128, 4, 6], F32, tag="st")
                mv = stats_pool.tile([128, 4, 2], F32, tag="mv")
                for g in range(4):
                    nc.vector.bn_stats(out=st[:, g, :], in_=qk[:, g, :])
                    nc.vector.bn_aggr(out=mv[:, g, :], in_=st[:, g, :])
                rstd = stats_pool.tile([128, 4], F32, tag="rstd")
                nc.scalar.activation(
                    out=rstd, in_=mv[:, :, 1], func=AF.Sqrt, bias=eps_sb
                )
                nc.vector.reciprocal(out=rstd, in_=rstd)
                # nmr = -mu * rstd (bias for the ACT-side LN apply)
                nmr = stats_pool.tile([128, 4], F32, tag="nmr")
                nc.vector.scalar_tensor_tensor(
                    out=nmr,
                    in0=mv[:, :, 0],
                    scalar=-1.0,
                    in1=rstd,
                    op0=ALU.mult,
                    op1=ALU.mult,
                )
                qn = stage1.tile([128, 128], BF16, tag="qn")
                kn = stage1.tile([128, 128], BF16, tag="kn")
                for g in range(4):
                    dst = qn if g < 2 else kn
                    dsl = dst[:, (g % 2) * HD : (g % 2 + 1) * HD]
                    if g < 2:
                        # q groups on ACT: (x - mu)*rstd == x*rstd + (-mu*rstd)
                        nc.scalar.activation(
                            out=dsl,
                            in_=qk[:, g, :],
                            func=AF.Identity,
                            scale=rstd[:, g : g + 1],
                            bias=nmr[:, g : g + 1],
                        )
                    else:
                        # k groups on DVE (balances ACT vs DVE load)
                        nc.vector.tensor_scalar(
                            out=dsl,
                            in0=qk[:, g, :],
                            scalar1=mv[:, g, 0:1],
                            scalar2=rstd[:, g : g + 1],
                            op0=ALU.subtract,
                            op1=ALU.mult,
                        )
                    if affine:
                        nc.vector.tensor_mul(dsl, dsl, gb_sb[:, 2 * (g // 2), :])
                        nc.vector.tensor_add(
                            dsl, dsl, gb_sb[:, 2 * (g // 2) + 1, :]
                        )
                # v (+ ones col already set)
                nc.vector.tensor_copy(
                    out=vO[:, :, tb, 0:HD],
                    in_=ps[:, 2 * DPC :].rearrange("p (h d) -> p h d", d=HD),
                )
                # transpose q,k into [dim, token] layout via DMA xbar
                ts = slice(tb * 128, (tb + 1) * 128)
                nc.sync.dma_start_transpose(out=qT[:, ts], in_=qn)
                nc.sync.dma_start_transpose(out=kT[:, ts], in_=kn)

        # ---------------- Phase 2: attention + projection ----------------
        with (
            tc.tile_pool(name="sc_ps", bufs=2, space="PSUM") as sc_ps,
            tc.tile_pool(name="o_ps", bufs=2, space="PSUM") as o_ps,
            tc.tile_pool(name="exps", bufs=4) as exps,
            tc.tile_pool(name="stage2", bufs=2) as stage2,
            tc.tile_pool(name="ostage", bufs=3) as ostage,
        ):
            ooms = {}

            def attn_chunk(ci):
                b, qc = divmod(ci, QC)
                cols = slice(b * S + qc * 512, b * S + (qc + 1) * 512)
                oom = o_ps.tile([HD + 1, HPC, 512], F32, tag="o", name="oom")
                ooms[ci] = oom
                for kb in range(KB):
                    gkb = b * KB + kb
                    ks = slice(gkb * 128, (gkb + 1) * 128)
                    # two heads' score matmuls live at partition bases
                    # 0/64 -> disjoint PE row groups run concurrently;
                    # one 1024-wide exp covers both heads
                    scp = sc_ps.tile(
                        [128, HPC, 512], F32, tag="s", name="scp"
                    )
                    for h in range(HPC):
                        hp = slice(h * HD, (h + 1) * HD)
                        nc.tensor.matmul(
                            scp[:, h, :],
                            lhsT=kT[hp, ks],
                            rhs=qT[hp, cols],
                            start=True,
                            stop=True,
                        )
                    ex = exps.tile(
                        [128, HPC, 512], BF16, tag="ex", name="ex"
                    )
                    nc.scalar.activation(
                        out=ex, in_=scp, func=AF.Exp, scale=SCALE
                    )
                    for h in range(HPC):
                        nc.tensor.matmul(
                            oom[:, h, :],
                            lhsT=vO[:, h, gkb, :],
                            rhs=ex[:, h, :],
                            start=(kb == 0),
                            stop=(kb == KB - 1),
                        )

            def epilogue_chunk(ci):
                b, qc = divmod(ci, QC)
                cols = slice(b * S + qc * 512, b * S + (qc + 1) * 512)
                oom = ooms.pop(ci)
                # 1/denominator on DVE (fast 18-bit custom op), broadcast
                # across partitions on the idle gpsimd engine (ACT stays
                # pure-Exp: no activation-table reloads in the phase)
                dn = stage2.tile([1, HPC, 512], F32, tag="dn", name="dn")
                nc.vector.tensor_copy(out=dn, in_=oom[HD : HD + 1, :, :])
                rc = stage2.tile([1, HPC, 512], F32, tag="rc", name="rc")
                nc.vector.reciprocal_approx_fast(out=rc, in_=dn)
                rbs = stage2.tile(
                    [128, HPC, 512], F32, tag="rbs", name="rbs"
                )
                nc.gpsimd.partition_broadcast(rbs, rc)
                for h in range(HPC):
                    nc.vector.tensor_mul(
                        aT[h * HD : (h + 1) * HD, cols],
                        oom[0:HD, h, :],
                        rbs[h * HD : (h + 1) * HD, h, :],
                    )
                # fused partial projection for the 4 token blocks of this
                # q-chunk; PSUM shares the score pool's banks and is evicted
                # to SBUF bf16 (projection bias is added on the host)
                for tbl in range(4):
                    tb = ci * 4 + tbl
                    rows = slice(tb * 128, (tb + 1) * 128)
                    ob = ostage.tile([128, D], BF16, tag="ob")
                    for nn in range(D // 512):
                        pps = sc_ps.tile(
                            [128, HPC, 512], F32, tag="s", name="pps"
                        )[:, 0, :]
                        nc.tensor.matmul(
                            pps,
                            lhsT=aT[:, rows],
                            rhs=wp_sb[:, nn * 512 : (nn + 1) * 512],
                            start=True,
                            stop=True,
                        )
                        nc.vector.tensor_copy(
                            out=ob[:, nn * 512 : (nn + 1) * 512], in_=pps
                        )
                    nc.sync.dma_start(out=outp[rows, :], in_=ob)

            # software pipeline: emit chunk ci's attention, then chunk
            # ci-1's epilogue+projection, so the in-order PE/ACT queues
            # always have ready attention work while the slow epilogue
            # dependency chain (denominator -> broadcast -> normalize)
            # resolves in the background
            NCHUNK = B * QC
            for ci in range(NCHUNK):
                attn_chunk(ci)
                if ci >= 1:
                    epilogue_chunk(ci - 1)
            epilogue_chunk(NCHUNK - 1)

    nc.compile()
    return nc


def make_in_maps(x, w_qkv, b_qkv, w_proj, q_gamma, q_beta, k_gamma, k_beta,
                 affine):
    B, S, _ = x.shape
    T = B * S
    xT = np.ascontiguousarray(x.reshape(T, D).T)
    import ml_dtypes
    bf = ml_dtypes.bfloat16
    in_maps = []
    for c in range(NCORES):
        rs = slice(c * DPC, (c + 1) * DPC)
        w_slice = np.concatenate(
            [w_qkv[rs], w_qkv[D:2 * D][rs.start:rs.stop], w_qkv[2 * D:][rs.start:rs.stop]],
            axis=0,
        )  # [384, 1024]
        b_slice = np.concatenate(
            [b_qkv[rs], b_qkv[D:2 * D][rs.start:rs.stop], b_qkv[2 * D:][rs.start:rs.stop]]
        )[None, :]  # [1, 384]
        m = {
            "xT": xT.astype(bf),
            "wt_qkv": np.ascontiguousarray(w_slice.T).astype(bf),
            "b_qkv_s": np.ascontiguousarray(b_slice).astype(bf),
            "wt_proj": np.ascontiguousarray(w_proj[:, rs].T).astype(bf),
            "c_onesb": np.ones((1, 128), bf),
        }
        if affine:
            gb = np.stack([q_gamma, q_beta, k_gamma, k_beta])  # [4, 64]
            m["c_gb"] = np.ascontiguousarray(
                np.broadcast_to(gb[None], (128, 4, HD)).astype(np.float32)
            )
        in_maps.append(m)
    return in_maps


_NC_CACHE = {}

LAST_RESULTS = None


def kernel(x, w_qkv, b_qkv, w_proj, b_proj, q_gamma, q_beta, k_gamma, k_beta,
           **unused):
    global LAST_RESULTS
    x = np.asarray(x, np.float32)
    w_qkv = np.asarray(w_qkv, np.float32)
    b_qkv = np.asarray(b_qkv, np.float32)
    w_proj = np.asarray(w_proj, np.float32)
    b_proj = np.asarray(b_proj, np.float32)
    q_gamma = np.asarray(q_gamma, np.float32)
    q_beta = np.asarray(q_beta, np.float32)
    k_gamma = np.asarray(k_gamma, np.float32)
    k_beta = np.asarray(k_beta, np.float32)

    B, S, _ = x.shape
    affine = not (
        np.all(q_gamma == 1) and np.all(k_gamma == 1)
        and np.all(q_beta == 0) and np.all(k_beta == 0)
    )
    key = (B, S, affine)
    if key not in _NC_CACHE:
        _NC_CACHE[key] = build_nc(B, S, affine)
    nc = _NC_CACHE[key]

    in_maps = make_in_maps(
        x, w_qkv, b_qkv, w_proj, q_gamma, q_beta, k_gamma, k_beta, affine
    )
    trace = bool(int(os.environ.get("BASS_KERNEL_TRACE", "0")))
    res = run_bass_kernel_spmd(
        nc, in_maps, core_ids=list(range(NCORES)), trace=trace
    )
    LAST_RESULTS = res
    acc = np.zeros((B * S, D), np.float32)
    for r in res.results:
        acc += np.asarray(r["outp"], np.float32)
    acc += b_proj[None, :]
    return acc.reshape(B, S, D)


# revision 19
# speedup vs baseline: 1.2308x; 1.0901x over previous
"""Multi-head self-attention (B=2, S=2048, D=1024, H=16) on 8 TRN2 NeuronCores.

Tensor-parallel over heads: each core owns 2 heads. Accepts FULL inputs,
returns FULL output. Host pre-transposes x and slices per-head weights;
each core computes qkv -> per-head LayerNorm -> attention -> partial
output projection (over its 128 embed dims); host sums the 8 partials
and adds the projection bias.
"""

import os
import sys

import numpy as np

for _p in ("/opt/trn_rl_repo", "/root/.axon_site/_ro/trn_rl_repo"):
    if os.path.isdir(_p) and _p not in sys.path:
        sys.path.insert(0, _p)
        break

import concourse.bass as bass  # noqa: E402
import concourse.bacc as bacc  # noqa: E402
import concourse.tile as tile  # noqa: E402
from concourse import mybir  # noqa: E402
from concourse.bass_utils import run_bass_kernel_spmd  # noqa: E402

F32 = mybir.dt.float32
F32R = mybir.dt.float32r
BF16 = mybir.dt.bfloat16
AF = mybir.ActivationFunctionType
ALU = mybir.AluOpType

NCORES = 8
D = 1024
H = 16
HD = 64
HPC = H // NCORES          # heads per core = 2
DPC = HPC * HD             # embed dims per core = 128
EPS = 1e-5


def build_nc(B, S, affine):
    """Build the SPMD Bass program for one core (same program, 8 cores)."""
    T = B * S                      # total token columns
    NTB = T // 128                 # 128-token blocks
    QC = S // 512                  # q-chunks per batch
    KB = S // 128                  # k-blocks per batch
    KCH = D // 128                 # contraction chunks (8)
    SCALE = 1.0 / np.sqrt(HD)

    nc = bacc.Bacc(
        "TRN2",
        target_bir_lowering=False,
        debug=False,
        enable_asserts=True,
        num_devices=NCORES,
    )

    xT = nc.dram_tensor("xT", [D, T], BF16, kind="ExternalInput").ap()
    wq = nc.dram_tensor("wt_qkv", [D, 3 * DPC], BF16, kind="ExternalInput").ap()
    bq = nc.dram_tensor("b_qkv_s", [1, 3 * DPC], BF16, kind="ExternalInput").ap()
    wp = nc.dram_tensor("wt_proj", [DPC, D], BF16, kind="ExternalInput").ap()
    onesb = nc.dram_tensor("c_onesb", [1, 128], BF16, kind="ExternalInput").ap()
    if affine:
        gb = nc.dram_tensor("c_gb", [128, 4, HD], F32, kind="ExternalInput").ap()
    outp = nc.dram_tensor("outp", [T, D], BF16, kind="ExternalOutput").ap()

    from contextlib import ExitStack

    with tile.TileContext(nc) as tc, ExitStack() as stack:
        const = stack.enter_context(tc.tile_pool(name="const", bufs=1))
        persist = stack.enter_context(tc.tile_pool(name="persist", bufs=1))

        wq_sb = const.tile([128, KCH, 3 * DPC], BF16, tag="wq")
        nc.sync.dma_start(
            out=wq_sb, in_=wq.rearrange("(c p) n -> p c n", p=128)
        )
        wp_sb = const.tile([DPC, D], BF16, tag="wp")
        nc.sync.dma_start(out=wp_sb, in_=wp)
        bq_sb = const.tile([1, 3 * DPC], BF16, tag="bq")
        nc.sync.dma_start(out=bq_sb, in_=bq)
        onesb_sb = const.tile([1, 128], BF16, tag="onesb")
        nc.sync.dma_start(out=onesb_sb, in_=onesb)
        eps_sb = const.tile([128, 1], F32, tag="eps")
        nc.vector.memset(eps_sb, EPS)

        if affine:
            gb_sb = const.tile([128, 4, HD], F32, tag="gb")
            nc.sync.dma_start(out=gb_sb, in_=gb)

        # whole x^T resident in SBUF; loaded in 8 token-chunks so the first
        # qkv block only waits on chunk 0
        xt_all = const.tile([128, KCH, T], BF16, tag="xt")
        for n in range(T // 512):
            nc.sync.dma_start(
                out=xt_all[:, :, n * 512 : (n + 1) * 512],
                in_=xT.rearrange("(c p) t -> p c t", p=128)[
                    :, :, n * 512 : (n + 1) * 512
                ],
            )

        # persistent intermediates
        qT = persist.tile([128, T], BF16, tag="qT")     # [2h*64, tok] LN'd q^T
        kT = persist.tile([128, T], BF16, tag="kT")
        vO = persist.tile([128, HPC, NTB, HD + 1], BF16, tag="vO")
        aT = persist.tile([128, T], BF16, tag="aT")     # attention out^T
        nc.vector.memset(vO[:, :, :, HD : HD + 1], 1.0)

        # ---------------- Phase 1: qkv + LayerNorm + transpose ----------
        with (
            tc.tile_pool(name="qkv_ps", bufs=6, space="PSUM") as qkv_ps,
            tc.tile_pool(name="stage1", bufs=6) as stage1,
            tc.tile_pool(name="stats", bufs=6) as stats_pool,
        ):
            for tb in range(NTB):
                ps = qkv_ps.tile([128, 3 * DPC], F32, tag="ps")
                nc.tensor.matmul(
                    ps,
                    lhsT=onesb_sb[0:1, 0:128],
                    rhs=bq_sb,
                    start=True,
                    stop=False,
                )
                for k in range(KCH):
                    nc.tensor.matmul(
                        ps,
                        lhsT=xt_all[:, k, tb * 128 : (tb + 1) * 128],
                        rhs=wq_sb[:, k, :],
                        start=False,
                        stop=(k == KCH - 1),
                    )
                # LayerNorm over each head's 64 dims of q and k
                qk = ps[:, 0 : 2 * DPC].rearrange("p (g d) -> p g d", d=HD)
                st = stats_pool.tile([128, 4, 6], F32, tag="st")
                mv = stats_pool.tile([128, 4, 2], F32, tag="mv")
                for g in range(4):
                    nc.vector.bn_stats(out=st[:, g, :], in_=qk[:, g, :])
                    nc.vector.bn_aggr(out=mv[:, g, :], in_=st[:, g, :])
                rstd = stats_pool.tile([128, 4], F32, tag="rstd")
                nc.scalar.activation(
                    out=rstd, in_=mv[:, :, 1], func=AF.Sqrt, bias=eps_sb
                )
                nc.vector.reciprocal(out=rstd, in_=rstd)
                # nmr = -mu * rstd (bias for the ACT-side LN apply)
                nmr = stats_pool.tile([128, 4], F32, tag="nmr")
                nc.vector.scalar_tensor_tensor(
                    out=nmr,
                    in0=mv[:, :, 0],
                    scalar=-1.0,
                    in1=rstd,
                    op0=ALU.mult,
                    op1=ALU.mult,
                )
                qn = stage1.tile([128, 128], BF16, tag="qn")
                kn = stage1.tile([128, 128], BF16, tag="kn")
                for g in range(4):
                    dst = qn if g < 2 else kn
                    dsl = dst[:, (g % 2) * HD : (g % 2 + 1) * HD]
                    if g < 2:
                        # q groups on ACT: (x - mu)*rstd == x*rstd + (-mu*rstd)
                        nc.scalar.activation(
                            out=dsl,
                            in_=qk[:, g, :],
                            func=AF.Identity,
                            scale=rstd[:, g : g + 1],
                            bias=nmr[:, g : g + 1],
                        )
                    else:
                        # k groups on DVE (balances ACT vs DVE load)
                        nc.vector.tensor_scalar(
                            out=dsl,
                            in0=qk[:, g, :],
                            scalar1=mv[:, g, 0:1],
                            scalar2=rstd[:, g : g + 1],
                            op0=ALU.subtract,
                            op1=ALU.mult,
                        )
                    if affine:
                        nc.vector.tensor_mul(dsl, dsl, gb_sb[:, 2 * (g // 2), :])
                        nc.vector.tensor_add(
                            dsl, dsl, gb_sb[:, 2 * (g // 2) + 1, :]
                        )
                # v (+ ones col already set)
                nc.vector.tensor_copy(
                    out=vO[:, :, tb, 0:HD],
                    in_=ps[:, 2 * DPC :].rearrange("p (h d) -> p h d", d=HD),
                )
                # transpose q,k into [dim, token] layout via DMA xbar
                ts = slice(tb * 128, (tb + 1) * 128)
                nc.sync.dma_start_transpose(out=qT[:, ts], in_=qn)
                nc.sync.dma_start_transpose(out=kT[:, ts], in_=kn)

        # ---------------- Phase 2: attention + projection ----------------
        with (
            tc.tile_pool(name="sc_ps", bufs=2, space="PSUM") as sc_ps,
            tc.tile_pool(name="o_ps", bufs=1, space="PSUM") as o_ps,
            tc.tile_pool(name="epi_ps", bufs=2, space="PSUM") as epi_ps,
            tc.tile_pool(name="exps", bufs=6) as exps,
            tc.tile_pool(name="stage2", bufs=2) as stage2,
            tc.tile_pool(name="ostage", bufs=3) as ostage,
        ):
            ooms = {}

            def attn_chunk(ci):
                b, qc = divmod(ci, QC)
                cols = slice(b * S + qc * 512, b * S + (qc + 1) * 512)
                oom = o_ps.tile([HD + 1, HPC, 512], F32, tag="o", name="oom")
                ooms[ci] = oom
                for kb in range(KB):
                    gkb = b * KB + kb
                    ks = slice(gkb * 128, (gkb + 1) * 128)
                    # two heads' score matmuls live at partition bases
                    # 0/64 -> disjoint PE row groups run concurrently;
                    # one 1024-wide exp covers both heads
                    scp = sc_ps.tile(
                        [128, HPC, 512], F32, tag="s", name="scp"
                    )
                    for h in range(HPC):
                        hp = slice(h * HD, (h + 1) * HD)
                        nc.tensor.matmul(
                            scp[:, h, :],
                            lhsT=kT[hp, ks],
                            rhs=qT[hp, cols],
                            start=True,
                            stop=True,
                        )
                    ex = exps.tile(
                        [128, HPC, 512], BF16, tag="ex", name="ex"
                    )
                    nc.scalar.activation(
                        out=ex, in_=scp, func=AF.Exp, scale=SCALE
                    )
                    for h in range(HPC):
                        nc.tensor.matmul(
                            oom[:, h, :],
                            lhsT=vO[:, h, gkb, :],
                            rhs=ex[:, h, :],
                            start=(kb == 0),
                            stop=(kb == KB - 1),
                        )

            def epilogue_chunk(ci):
                b, qc = divmod(ci, QC)
                cols = slice(b * S + qc * 512, b * S + (qc + 1) * 512)
                oom = ooms.pop(ci)
                # evacuate the PSUM accumulator early (raw attention sums +
                # denominator row) so the next chunk's attnv can reuse the
                # single oom buffer without waiting on the normalize chain
                dn = stage2.tile([1, HPC, 512], F32, tag="dn", name="dn")
                nc.vector.tensor_copy(out=dn, in_=oom[HD : HD + 1, :, :])
                # per-head staging at matching partition offsets (the BIR
                # verifier requires SBUF operands on identical partitions;
                # only the PSUM side may shift)
                av = stage2.tile([128, 512], F32, tag="av", name="av")
                for h in range(HPC):
                    nc.vector.tensor_copy(
                        out=av[h * HD : (h + 1) * HD, :], in_=oom[0:HD, h, :]
                    )
                # 1/denominator on DVE (fast 18-bit custom op), broadcast
                # across partitions on the idle gpsimd engine (ACT stays
                # pure-Exp: no activation-table reloads in the phase)
                rc = stage2.tile([1, HPC, 512], F32, tag="rc", name="rc")
                nc.vector.reciprocal_approx_fast(out=rc, in_=dn)
                rbs = stage2.tile(
                    [128, HPC, 512], F32, tag="rbs", name="rbs"
                )
                nc.gpsimd.partition_broadcast(rbs, rc)
                for h in range(HPC):
                    hp = slice(h * HD, (h + 1) * HD)
                    nc.vector.tensor_mul(
                        aT[hp, cols], av[hp, :], rbs[hp, h, :]
                    )
                # fused partial projection for the 4 token blocks of this
                # q-chunk; PSUM evicted to SBUF bf16 (projection bias is
                # added on the host)
                for tbl in range(4):
                    tb = ci * 4 + tbl
                    rows = slice(tb * 128, (tb + 1) * 128)
                    ob = ostage.tile([128, D], BF16, tag="ob")
                    for nn in range(D // 512):
                        pps = epi_ps.tile(
                            [128, 512], F32, tag="pps", name="pps"
                        )
                        nc.tensor.matmul(
                            pps,
                            lhsT=aT[:, rows],
                            rhs=wp_sb[:, nn * 512 : (nn + 1) * 512],
                            start=True,
                            stop=True,
                        )
                        nc.vector.tensor_copy(
                            out=ob[:, nn * 512 : (nn + 1) * 512], in_=pps
                        )
                    nc.sync.dma_start(out=outp[rows, :], in_=ob)

            # software pipeline: emit chunk ci's attention, then chunk
            # ci-1's epilogue+projection, so the in-order PE/ACT queues
            # always have ready attention work while the slow epilogue
            # dependency chain (denominator -> broadcast -> normalize)
            # resolves in the background
            NCHUNK = B * QC
            for ci in range(NCHUNK):
                attn_chunk(ci)
                if ci >= 1:
                    epilogue_chunk(ci - 1)
            epilogue_chunk(NCHUNK - 1)

    nc.compile()
    return nc


def make_in_maps(x, w_qkv, b_qkv, w_proj, q_gamma, q_beta, k_gamma, k_beta,
                 affine):
    B, S, _ = x.shape
    T = B * S
    xT = np.ascontiguousarray(x.reshape(T, D).T)
    import ml_dtypes
    bf = ml_dtypes.bfloat16
    in_maps = []
    for c in range(NCORES):
        rs = slice(c * DPC, (c + 1) * DPC)
        w_slice = np.concatenate(
            [w_qkv[rs], w_qkv[D:2 * D][rs.start:rs.stop], w_qkv[2 * D:][rs.start:rs.stop]],
            axis=0,
        )  # [384, 1024]
        b_slice = np.concatenate(
            [b_qkv[rs], b_qkv[D:2 * D][rs.start:rs.stop], b_qkv[2 * D:][rs.start:rs.stop]]
        )[None, :]  # [1, 384]
        m = {
            "xT": xT.astype(bf),
            "wt_qkv": np.ascontiguousarray(w_slice.T).astype(bf),
            "b_qkv_s": np.ascontiguousarray(b_slice).astype(bf),
            "wt_proj": np.ascontiguousarray(w_proj[:, rs].T).astype(bf),
            "c_onesb": np.ones((1, 128), bf),
        }
        if affine:
            gb = np.stack([q_gamma, q_beta, k_gamma, k_beta])  # [4, 64]
            m["c_gb"] = np.ascontiguousarray(
                np.broadcast_to(gb[None], (128, 4, HD)).astype(np.float32)
            )
        in_maps.append(m)
    return in_maps


_NC_CACHE = {}

LAST_RESULTS = None


def kernel(x, w_qkv, b_qkv, w_proj, b_proj, q_gamma, q_beta, k_gamma, k_beta,
           **unused):
    global LAST_RESULTS
    x = np.asarray(x, np.float32)
    w_qkv = np.asarray(w_qkv, np.float32)
    b_qkv = np.asarray(b_qkv, np.float32)
    w_proj = np.asarray(w_proj, np.float32)
    b_proj = np.asarray(b_proj, np.float32)
    q_gamma = np.asarray(q_gamma, np.float32)
    q_beta = np.asarray(q_beta, np.float32)
    k_gamma = np.asarray(k_gamma, np.float32)
    k_beta = np.asarray(k_beta, np.float32)

    B, S, _ = x.shape
    affine = not (
        np.all(q_gamma == 1) and np.all(k_gamma == 1)
        and np.all(q_beta == 0) and np.all(k_beta == 0)
    )
    key = (B, S, affine)
    if key not in _NC_CACHE:
        _NC_CACHE[key] = build_nc(B, S, affine)
    nc = _NC_CACHE[key]

    in_maps = make_in_maps(
        x, w_qkv, b_qkv, w_proj, q_gamma, q_beta, k_gamma, k_beta, affine
    )
    trace = bool(int(os.environ.get("BASS_KERNEL_TRACE", "0")))
    res = run_bass_kernel_spmd(
        nc, in_maps, core_ids=list(range(NCORES)), trace=trace
    )
    LAST_RESULTS = res
    acc = np.zeros((B * S, D), np.float32)
    for r in res.results:
        acc += np.asarray(r["outp"], np.float32)
    acc += b_proj[None, :]
    return acc.reshape(B, S, D)
